# revision 1
# baseline (speedup 1.0000x reference)
"""Trainium2 Bass kernel for nn_CLIP_3v3d_brats (dense_cnn head + gated 1x1 conv).

Sharding: 8 cores = batch(2) x 4 D-slabs of `pred`. The dominant einsum
logits[b,k,:] = sum_c effw[b,k,c]*pred[b,c,:] runs as a block-diagonal
float32r matmul (4 position groups -> K=128, M=12, ~1 cycle/row).

GAP head: conv+global-mean collapse to x_feat = W2d @ S where S are 27
strided window sums of relu(groupnorm(x_e)). Head work is sharded by
CHANNELS (16 ch/core = exactly 2 GroupNorm groups, so stats stay
core-local); window sums use a host-gathered (channel, offset-group)
partition packing with ReLU+normalize+window-sum fused into single ACT
instructions via accum_out. ONE AllReduce total (x_feat partials); a
dep-free warmup collective absorbs part of the mesh cold-start. The
collective fabric has a fixed ~60-100us cold-start from kernel launch
(cross-core skew), so the stream prefetches ~13.5MB of pred into SBUF
during that window; the remaining stream runs DMA-saturated.
"""
import sys
import types

sys.path.insert(0, "/opt/trn_rl_repo")

import numpy as np

# Register the NTFF profile hook the agent image's antenv lacks (only
# needed when TRACE is enabled; harmless otherwise).
try:
    import antenv.axon_hooks  # noqa: F401
except ImportError:
    try:
        import trn_agent_boot.trn_boot as _tb

        _hooks = types.ModuleType("antenv.axon_hooks")
        _the_hook = _tb._ntff_profile_via_ctypes("/opt/axon/libaxon_pjrt.so")
        _hooks.get_axon_ntff_profile_hook = lambda: _the_hook
        _hooks.set_axon_ntff_profile_hook = lambda h: None
        sys.modules["antenv.axon_hooks"] = _hooks
    except Exception:
        pass

from concourse import bacc, tile, mybir
from concourse.bass_utils import run_bass_kernel_spmd

f32 = mybir.dt.float32
f32r = mybir.dt.float32r
AF = mybir.ActivationFunctionType
ALU = mybir.AluOpType

N_CORES = 8
B = 2
K = 3
EPS = 1e-5
G = 4                      # position groups interleaved on partitions
NPOS = 221184              # positions per core slab: 24*96*96
NG = NPOS // G             # 55296
COLS = 4608                # stream iteration columns (9 matmuls of 512)
NITER = NG // COLS         # 12
NMM = COLS // 512          # 9
NSLOT = 4                  # window-sum slots per core
NWIN = 1331                # 11^3 window positions per offset
NSLAB = 1728               # 3*24*24 stats-slab positions per batch
NGRP_ELEMS = 8 * 13824     # elements per (batch, group) for GN stats

TRACE = False
LAST_EXEC_NS = None
_CACHE = {}


def _build_program():
    nc = bacc.Bacc("TRN2", target_bir_lowering=False, debug=False,
                   num_devices=N_CORES)

    def din(name, shape, dt=f32):
        return nc.dram_tensor(name, shape, dt, kind="ExternalInput").ap()

    pred_s = din("pred_s", [NITER, 128, COLS], f32r)
    xe_slab_d = din("xe_slab", [128, B * NSLAB])
    xg_d = din("xg", [128, B * NSLOT * NWIN])
    w2dt_d = din("w2dt", [128, NSLOT * 256])
    gstat_d = din("gstat", [128, 2])
    gexp_d = din("gexp", [2, 128])
    gnw_d = din("gnw", [128, 2])
    w_cfT_d = din("w_cfT", [128, 2 * 512])
    bcf6_d = din("bcf6", [6, 4 * 128])
    id6_d = din("id6", [6, 6])
    w_cT_d = din("w_cT", [128, 4 * 256])
    bcT_d = din("bcT", [128, 2])
    w_a1T_d = din("w_a1T", [128, 2 * 16])
    ba1_d = din("ba1", [16, 1])
    w_a2T_d = din("w_a2T", [16, 32])
    ba2_d = din("ba2", [32, 1])
    wseg6_d = din("wseg6", [32, 6])
    msel_d = din("msel", [32, 2])
    gapbT_d = din("gapbT", [128, 4])
    ones6_d = din("ones6", [128, 6])
    bseg12_d = din("bseg12", [12, 1])

    out_d = nc.dram_tensor("out_s", [12, NG], f32,
                           kind="ExternalOutput").ap()

    with tile.TileContext(nc) as tc:
        with tc.tile_pool(name="small", bufs=1) as sp, \
             tc.tile_pool(name="pred", bufs=6) as pp, \
             tc.tile_pool(name="outp", bufs=2) as op, \
             tc.tile_pool(name="hps", bufs=3, space="PSUM") as hps, \
             tc.tile_pool(name="sps", bufs=4, space="PSUM") as sps, \
             tc.tile_pool(name="dram", bufs=1, space="DRAM") as dram:
          with tc.tile_pool(name="headbig", bufs=1) as hb, \
               tc.tile_pool(name="scratch", bufs=2) as scp:

            # ---- head constant loads (emitted first: priority on DMA) ----
            xe_slab = hb.tile([128, B * NSLAB], f32)
            nc.sync.dma_start(xe_slab[:], xe_slab_d[:])
            gstat = sp.tile([128, 2], f32)
            nc.sync.dma_start(gstat[:], gstat_d[:])
            gexp = sp.tile([2, 128], f32)
            nc.sync.dma_start(gexp[:], gexp_d[:])
            gnw = sp.tile([128, 2], f32)
            nc.sync.dma_start(gnw[:], gnw_d[:])
            xgs = []
            for col in range(B * NSLOT):
                xgt = scp.tile([128, NWIN], f32, tag="xgs", bufs=2)
                nc.sync.dma_start(
                    xgt[:], xg_d[:, col * NWIN:(col + 1) * NWIN])
                xgs.append(xgt)
            w2dt = hb.tile([128, NSLOT * 256], f32)
            nc.sync.dma_start(w2dt[:], w2dt_d[:])
            w_cfT = hb.tile([128, 2 * 512], f32)
            nc.sync.dma_start(w_cfT[:], w_cfT_d[:])
            bcf6 = sp.tile([6, 4 * 128], f32)
            nc.sync.dma_start(bcf6[:], bcf6_d[:])
            id6 = sp.tile([6, 6], f32)
            nc.sync.dma_start(id6[:], id6_d[:])
            w_cT = hb.tile([128, 4 * 256], f32)
            nc.sync.dma_start(w_cT[:], w_cT_d[:])
            bcT = sp.tile([128, 2], f32)
            nc.sync.dma_start(bcT[:], bcT_d[:])
            w_a1T = sp.tile([128, 2 * 16], f32)
            nc.sync.dma_start(w_a1T[:], w_a1T_d[:])
            ba1 = sp.tile([16, 1], f32)
            nc.sync.dma_start(ba1[:], ba1_d[:])
            w_a2T = sp.tile([16, 32], f32)
            nc.sync.dma_start(w_a2T[:], w_a2T_d[:])
            ba2 = sp.tile([32, 1], f32)
            nc.sync.dma_start(ba2[:], ba2_d[:])
            wseg6 = sp.tile([32, 6], f32)
            nc.sync.dma_start(wseg6[:], wseg6_d[:])
            msel = sp.tile([32, 2], f32)
            nc.sync.dma_start(msel[:], msel_d[:])
            gapbT = sp.tile([128, 4], f32)
            nc.sync.dma_start(gapbT[:], gapbT_d[:])
            ones6 = sp.tile([128, 6], f32)
            nc.sync.dma_start(ones6[:], ones6_d[:])
            bseg12 = sp.tile([12, 1], f32)
            nc.sync.dma_start(bseg12[:], bseg12_d[:])

            # ---- collective-fabric warmup (no data deps) ----
            warm_in = dram.tile([2, 2], f32)
            warm_out = dram.tile([2, 2], f32)
            nc.gpsimd.collective_compute(
                "AllReduce", ALU.add,
                replica_groups=[list(range(N_CORES))],
                ins=[warm_in.opt()], outs=[warm_out.opt()])

            # ---- GN stats (core-local: 16 channels = 2 full groups) ----
            stat4 = sp.tile([128, 4], f32)  # cols: 2*b + (0=sum, 1=sumsq)
            for b in range(B):
                sl = xe_slab[:, b * NSLAB:(b + 1) * NSLAB]
                st_sc = scp.tile([128, NSLAB], f32, tag="sc", bufs=1)
                nc.scalar.activation(st_sc[:], sl, AF.Copy,
                                     accum_out=stat4[:, 2 * b:2 * b + 1])
                st_sc2 = scp.tile([128, NSLAB], f32, tag="sc", bufs=1)
                nc.scalar.activation(st_sc2[:], sl, AF.Square,
                                     accum_out=stat4[:, 2 * b + 1:2 * b + 2])

            # group-sum via mask matmul: [2, 4] (both groups are core-local)
            g4 = hps.tile([2, 4], f32, tag="hps")
            nc.tensor.matmul(g4[:], gstat[:], stat4[:], start=True, stop=True)
            gsum = sp.tile([2, 4], f32)
            nc.vector.tensor_copy(gsum[:], g4[:])

            # mu(neg), rsqrt(var+eps) per (group, b) -> mr4 [2,4]
            mr4 = sp.tile([2, 4], f32)  # cols: -mu0, -mu1, rs0, rs1
            nc.scalar.mul(mr4[:, 0:2], gsum[:, 0:4:2], -1.0 / NGRP_ELEMS)
            ex2 = sp.tile([2, 2], f32)
            nc.scalar.mul(ex2[:], gsum[:, 1:4:2], 1.0 / NGRP_ELEMS)
            musq = sp.tile([2, 2], f32)
            nc.vector.tensor_mul(musq[:], mr4[:, 0:2], mr4[:, 0:2])
            var = sp.tile([2, 2], f32)
            nc.vector.tensor_sub(var[:], ex2[:], musq[:])
            vare = sp.tile([2, 2], f32)
            nc.vector.tensor_scalar_add(vare[:], var[:], float(EPS))
            sd = sp.tile([2, 2], f32)
            nc.scalar.activation(sd[:], vare[:], AF.Sqrt)
            nc.vector.reciprocal(mr4[:, 2:4], sd[:])

            # expand groups -> (c,og) partitions: chmr [128,4]
            ch4 = hps.tile([128, 4], f32, tag="hps")
            nc.tensor.matmul(ch4[:], gexp[:], mr4[:], start=True, stop=True)
            chmr = sp.tile([128, 4], f32)
            nc.vector.tensor_copy(chmr[:], ch4[:])
            # scale_c = rs*gamma ; bias_c = beta + (-mu)*scale
            scale = sp.tile([128, 2], f32)
            nc.vector.tensor_scalar_mul(scale[:], chmr[:, 2:4], gnw[:, 0:1])
            nmus = sp.tile([128, 2], f32)
            nc.vector.tensor_mul(nmus[:], chmr[:, 0:2], scale[:])
            bias = sp.tile([128, 2], f32)
            nc.vector.tensor_scalar_add(bias[:], nmus[:], gnw[:, 1:2])

            # ---- fused relu-normalize + window-sum into S4 [128, 8] ----
            S4 = sp.tile([128, B * NSLOT], f32)
            for b in range(B):
                for s in range(NSLOT):
                    col = b * NSLOT + s
                    rl_sc = scp.tile([128, NWIN], f32, tag="sc", bufs=1)
                    nc.scalar.activation(
                        rl_sc[:], xgs[col][:], AF.Relu,
                        bias=bias[:, b:b + 1], scale=scale[:, b:b + 1],
                        accum_out=S4[:, col:col + 1])

            # ---- x_feat partials: xfT chunks [128, 2] via W2dT matmuls ----
            xfs = sp.tile([128, 4], f32)  # cols: oc*2 + b
            for oc in range(2):
                xfp = hps.tile([128, 2], f32, tag="hps")
                for s in range(NSLOT):
                    nc.tensor.matmul(
                        xfp[:],
                        w2dt[:, s * 256 + oc * 128: s * 256 + oc * 128 + 128],
                        S4[:, s:s + NSLOT + 1:NSLOT],
                        start=(s == 0), stop=(s == NSLOT - 1))
                nc.vector.tensor_copy(xfs[:, oc * 2:oc * 2 + 2], xfp[:])

            ar2_in = dram.tile([128, 4], f32)
            ar2_out = dram.tile([128, 4], f32)
            nc.gpsimd.dma_start(ar2_in[:], xfs[:])
            nc.gpsimd.collective_compute(
                "AllReduce", ALU.add,
                replica_groups=[list(range(N_CORES))],
                ins=[ar2_in.opt()], outs=[ar2_out.opt()])
            xfr = sp.tile([128, 4], f32)
            nc.gpsimd.dma_start(xfr[:], ar2_out[:])
            xfb = sp.tile([128, 4], f32)
            nc.vector.tensor_add(xfb[:], xfr[:], gapbT[:])

            # ---- xcT for feature half: [128, 12] cols pc*6 + (3b+k) ----
            xcT = sp.tile([128, 12], f32)
            for pc in range(2):
                for b in range(B):
                    nc.vector.tensor_scalar_mul(
                        xcT[:, pc * 6 + 3 * b: pc * 6 + 3 * b + 3],
                        ones6[:, 0:3],
                        xfb[:, pc * 2 + b: pc * 2 + b + 1])

            # ---- MLP1: p6T = relu(Wx @ x_feat + (We@emb + b_cf)).T ----
            p6T = sp.tile([128, 4 * 6], f32)
            for oc in range(4):
                p1 = hps.tile([128, 6], f32, tag="hps")
                for pc in range(2):
                    nc.tensor.matmul(
                        p1[:],
                        w_cfT[:, pc * 512 + oc * 128: pc * 512 + oc * 128 + 128],
                        xcT[:, pc * 6:pc * 6 + 6],
                        start=(pc == 0), stop=False)
                nc.tensor.matmul(p1[:], bcf6[:, oc * 128:(oc + 1) * 128],
                                 id6[:], start=False, stop=True)
                nc.scalar.activation(p6T[:, oc * 6:oc * 6 + 6], p1[:], AF.Relu)

            # ---- MLP2: c6T [128, 2*6] ----
            c6T = sp.tile([128, 2 * 6], f32)
            for oc in range(2):
                c1 = hps.tile([128, 6], f32, tag="hps")
                for pc in range(4):
                    nc.tensor.matmul(
                        c1[:],
                        w_cT[:, pc * 256 + oc * 128: pc * 256 + oc * 128 + 128],
                        p6T[:, pc * 6:pc * 6 + 6],
                        start=(pc == 0), stop=(pc == 3))
                nc.scalar.activation(c6T[:, oc * 6:oc * 6 + 6], c1[:],
                                     AF.Identity, bias=bcT[:, oc:oc + 1])

            # ---- MLP3: hT [16, 6] ----
            h1 = hps.tile([16, 6], f32, tag="hps")
            for pc in range(2):
                nc.tensor.matmul(h1[:], w_a1T[:, pc * 16:pc * 16 + 16],
                                 c6T[:, pc * 6:pc * 6 + 6],
                                 start=(pc == 0), stop=(pc == 1))
            hT = sp.tile([16, 6], f32)
            nc.scalar.activation(hT[:], h1[:], AF.Relu, bias=ba1[:, 0:1])

            # ---- MLP4: gT [32, 6] = sigmoid(...) ----
            g1 = hps.tile([32, 6], f32, tag="hps")
            nc.tensor.matmul(g1[:], w_a2T[:], hT[:], start=True, stop=True)
            gT = sp.tile([32, 6], f32)
            nc.scalar.activation(gT[:], g1[:], AF.Sigmoid, bias=ba2[:, 0:1])

            # ---- effw + batch select + block-diagonal lhsT [128, 12] ----
            effT = sp.tile([32, 6], f32)
            nc.vector.tensor_mul(effT[:], gT[:], wseg6[:])
            selL = sp.tile([32, 3], f32)
            nc.vector.tensor_scalar_mul(selL[:], effT[:, 0:3], msel[:, 0:1])
            selR = sp.tile([32, 3], f32)
            nc.vector.tensor_scalar_mul(selR[:], effT[:, 3:6], msel[:, 1:2])
            effB = sp.tile([32, 3], f32)
            nc.vector.tensor_add(effB[:], selL[:], selR[:])

            bd = sp.tile([128, 12], f32r)
            nc.vector.memset(bd[:].bitcast(mybir.dt.uint32), 0)
            for g in range(G):
                nc.sync.dma_start(bd[32 * g:32 * g + 32, 3 * g:3 * g + 3],
                                  effB[:].bitcast(f32r))

          # ---- main stream: 12 x (one 2.25MB DMA -> 9 matmuls -> out) ----
          for t in range(NITER):
            pt = pp.tile([128, COLS], f32r, tag="pt")
            for g in range(G):
                nc.sync.dma_start(pt[32 * g:32 * g + 32, :],
                                  pred_s[t, 32 * g:32 * g + 32, :])
            so = op.tile([12, COLS], f32, tag="so")
            for m in range(NMM):
                po = sps.tile([12, 512], f32, tag="po")
                nc.tensor.matmul(po[:], bd[:], pt[:, m * 512:(m + 1) * 512],
                                 start=True, stop=True)
                if m % 2 == 0:
                    nc.scalar.activation(so[:, m * 512:(m + 1) * 512], po[:],
                                         AF.Identity, bias=bseg12[:, 0:1])
                else:
                    nc.vector.tensor_scalar_add(
                        so[:, m * 512:(m + 1) * 512], po[:], bseg12[:, 0:1])
            nc.gpsimd.dma_start(out_d[:, t * COLS:(t + 1) * COLS], so[:])

    nc.compile()
    return nc


def _prep_shared(inp):
    """Host-side weight transposes shared by all cores."""
    gn_g = np.asarray(inp["gn_g"], np.float32)
    gn_b = np.asarray(inp["gn_b"], np.float32)
    gap_b = np.asarray(inp["gap_b"], np.float32)
    w_cf = np.asarray(inp["w_cf"], np.float32)
    b_cf = np.asarray(inp["b_cf"], np.float32)
    w_c = np.asarray(inp["w_c"], np.float32)
    b_c = np.asarray(inp["b_c"], np.float32)
    w_a1 = np.asarray(inp["w_a1"], np.float32)
    b_a1 = np.asarray(inp["b_a1"], np.float32)
    w_a2 = np.asarray(inp["w_a2"], np.float32)
    b_a2 = np.asarray(inp["b_a2"], np.float32)
    emb = np.asarray(inp["emb"], np.float32)
    w_seg = np.asarray(inp["w_seg"], np.float32)
    b_seg = np.asarray(inp["b_seg"], np.float32)

    p = np.arange(128)
    gstat = (p[:, None] // 64 == np.arange(2)[None, :]).astype(np.float32)
    gexp = np.ascontiguousarray(gstat.T)

    # x-half of w_cf, transposed: [128, 2*512]
    wx = w_cf[:, 0:256].T                            # [256, 512]
    w_cfT = np.concatenate(
        [wx[128 * pc:128 * (pc + 1), :] for pc in range(2)], axis=1)
    # constant-folded emb-half + bias: bcf6[r, o] = b_cf[o] + We @ emb
    j = np.arange(6)
    bcf6 = np.ascontiguousarray(
        b_cf[None, :] + emb[j % 3] @ w_cf[:, 256:512].T)  # [6, 512]
    id6 = np.eye(6, dtype=np.float32)
    w_cT = np.concatenate(
        [w_c.T[128 * pc:128 * (pc + 1), :] for pc in range(4)], axis=1)
    bcT = np.ascontiguousarray(b_c.reshape(2, 128).T)
    w_a1T = np.concatenate(
        [w_a1.T[128 * pc:128 * (pc + 1), :] for pc in range(2)], axis=1)
    ba1 = b_a1.reshape(16, 1)
    w_a2T = np.ascontiguousarray(w_a2.T)
    ba2 = b_a2.reshape(32, 1)

    wseg6 = np.ascontiguousarray(w_seg[j % 3, :].T)
    gapbT = np.ascontiguousarray(
        np.repeat(gap_b.reshape(2, 128).T, 2, axis=1))  # cols oc*2+b
    ones6 = np.ones((128, 6), np.float32)
    bseg12 = np.tile(b_seg, 4).reshape(12, 1)

    return dict(gstat=gstat, gexp=gexp, w_cfT=w_cfT, bcf6=bcf6, id6=id6,
                w_cT=w_cT, bcT=bcT, w_a1T=w_a1T, ba1=ba1, w_a2T=w_a2T,
                ba2=ba2, wseg6=wseg6, gapbT=gapbT, ones6=ones6,
                bseg12=bseg12)


def kernel(**inputs):
    global LAST_EXEC_NS
    x_e = np.asarray(inputs["x_e"], np.float32)
    pred = np.asarray(inputs["pred"], np.float32)
    gap_w = np.asarray(inputs["gap_w"], np.float32)
    gn_g = np.asarray(inputs["gn_g"], np.float32)
    gn_b = np.asarray(inputs["gn_b"], np.float32)

    shared = _prep_shared(inputs)
    shared = {k: np.ascontiguousarray(v, dtype=np.float32)
              for k, v in shared.items()}

    # (og, s) -> conv offset table, identical on every core
    offs = [(4 * og + s) % 27 for og in range(8) for s in range(NSLOT)]
    cnt = np.bincount(np.array(offs), minlength=27).astype(np.float32)
    w2 = gap_w.reshape(256, 128, 27)

    # all 27 strided windows of x_e, gathered once: [27, B, 128, NWIN]
    wins = np.empty((27, B, 128, NWIN), np.float32)
    for o in range(27):
        kd, kw, kh = o // 9, (o // 3) % 3, o % 3
        win = x_e[:, :, kd:kd + 21:2, kw:kw + 21:2, kh:kh + 21:2]
        wins[o] = win.reshape(B, 128, NWIN)

    in_maps = []
    for r in range(N_CORES):
        b, dq = divmod(r, 4)
        m = dict(shared)
        ch = slice(16 * r, 16 * r + 16)

        ps = pred[b, :, dq * 24:(dq + 1) * 24]          # [32,24,96,96]
        ps = ps.reshape(32, G, NITER, COLS).transpose(2, 1, 0, 3)
        m["pred_s"] = np.ascontiguousarray(ps.reshape(NITER, 128, COLS))

        # stats slab: partitions (c:16, dchunk:8), cols b*1728 + pos
        sl = x_e[:, ch].reshape(B, 16, 8, NSLAB)
        m["xe_slab"] = np.ascontiguousarray(
            sl.transpose(1, 2, 0, 3).reshape(128, -1))

        # window gather: partitions (c:16, og:8), cols (b, s, pos)
        xgr = np.empty((16, 8, B, NSLOT, NWIN), np.float32)
        w2dt = np.empty((16, 8, NSLOT, 256), np.float32)
        for og in range(8):
            for sidx in range(NSLOT):
                o = offs[og * NSLOT + sidx]
                xgr[:, og, :, sidx, :] = wins[o][:, ch].transpose(1, 0, 2)
                w2dt[:, og, sidx, :] = (
                    w2[:, ch, o].T / np.float32(1331.0 * cnt[o]))
        m["xg"] = np.ascontiguousarray(xgr.reshape(128, -1))
        m["w2dt"] = np.ascontiguousarray(w2dt.reshape(128, -1))

        # per-(c,og) gamma/beta
        m["gnw"] = np.ascontiguousarray(
            np.stack([np.repeat(gn_g[ch], 8), np.repeat(gn_b[ch], 8)],
                     axis=1))

        msel = np.zeros((32, 2), np.float32)
        msel[:, b] = 1.0
        m["msel"] = msel
        in_maps.append(m)

    if "nc" not in _CACHE:
        _CACHE["nc"] = _build_program()
    nc = _CACHE["nc"]

    res = run_bass_kernel_spmd(nc, in_maps, list(range(N_CORES)),
                               trace=TRACE)
    LAST_EXEC_NS = res.exec_time_ns

    out = np.empty((B, K, 96, 96, 96), np.float32)
    for r in range(N_CORES):
        b, dq = divmod(r, 4)
        o = res.results[r]["out_s"]                      # [12, NG]
        o = o.reshape(G, K, NG).transpose(1, 0, 2).reshape(K, NPOS)
        out[b, :, dq * 24:(dq + 1) * 24] = o.reshape(K, 24, 96, 96)
    return out



# revision 23
# speedup vs baseline: 1.6121x; 1.6121x over previous
"""Trainium2 Bass kernel for nn_CLIP_3v3d_brats (dense_cnn head + gated 1x1 conv).

Sharding: 8 cores = batch(2) x 4 D-slabs of `pred`. The dominant einsum
logits[b,k,:] = sum_c effw[b,k,c]*pred[b,c,:] runs as a block-diagonal
bf16 matmul (4 position groups -> K=128 contraction, M=12 out rows).

pred is cast to bf16 on host (halves the dominant HBM stream; rel err
~2e-3 vs 2e-2 budget). GAP head is sharded by CHANNELS (16 ch/core = 2
GroupNorm groups, stats core-local). Window sums are computed ON DEVICE
from the raw 1.77MB x_e slab via a parity-packed layout: partitions =
(16ch x 8 parity-classes), each parity class holds a 12^3 subcube, and
one strided reduce over [11,11,11] windows at local start (sd,sw,sh) in
{0,1}^3 yields, per partition, the window sum for conv offset
(2s+parity) -- 8 reduce instructions x 2 batches cover all 27 offsets
with all 128 partitions busy. The AllReduce payload is the FIRST MLP
layer's linear output z = Wx @ x_feat_partial [128,8] so the matmul and
the gap_b bias-add are folded in before the collective. All small
weights travel in ONE packed [128, ~4300] tensor (one DMA). pred goes
in 3 big column-chunk DMAs of one [128, 55296] bf16 tensor.
"""
import sys
import types

sys.path.insert(0, "/opt/trn_rl_repo")

import numpy as np
import ml_dtypes

# Register the NTFF profile hook the agent image's antenv lacks (only
# needed when TRACE is enabled; harmless otherwise).
try:
    import antenv.axon_hooks  # noqa: F401
except ImportError:
    try:
        import trn_agent_boot.trn_boot as _tb

        _hooks = types.ModuleType("antenv.axon_hooks")
        _the_hook = _tb._ntff_profile_via_ctypes("/opt/axon/libaxon_pjrt.so")
        _hooks.get_axon_ntff_profile_hook = lambda: _the_hook
        _hooks.set_axon_ntff_profile_hook = lambda h: None
        sys.modules["antenv.axon_hooks"] = _hooks
    except Exception:
        pass

from concourse import bacc, tile, mybir
from concourse.bass_utils import run_bass_kernel_spmd

f32 = mybir.dt.float32
bf16 = mybir.dt.bfloat16
AF = mybir.ActivationFunctionType
ALU = mybir.AluOpType
AXL = mybir.AxisListType
bfloat16 = ml_dtypes.bfloat16

N_CORES = 8
B = 2
K = 3
EPS = 1e-5
G = 4                      # position groups interleaved on partitions
NPOS = 221184              # positions per core slab: 24*96*96
NG = NPOS // G             # 55296
COLS = 4608                # out-chunk columns (9 matmuls of 512)
NITER = NG // COLS         # 9
NMM = COLS // 512          # 12
NCHUNK = 3                 # pred DMA chunks
CHW = NG // NCHUNK         # 18432 cols per pred DMA
NGRP_ELEMS = 8 * 13824     # elements per (batch, group) for GN stats

# ---- packed-weights column layout (f32, 128 partitions) ----
_WL = [
    ("w2dt", 8 * 2 * 128),   # [s, m, occ]  xf lhsT slots
    ("wx",   2 * 4 * 128),   # [pc, m, occ] z = Wx @ xf lhsT slots
    ("wc",   4 * 2 * 128),   # [pc, m2, occ] c = Wc @ p lhsT slots
    ("wa1",  2 * 16),        # [pc2, 16]
    ("wa24", 128),           # [16, 128] on partitions 0..15
    ("bcf",  4 * 3 * 2),     # [m, k, b] fused bias for p-layer
    ("gstat", 2),            # [128, 2] partition->group mask
    ("gexp", 128),           # [2, 128] on partitions 0..1
    ("gnw",  2),             # gamma/beta per partition
    ("bct",  2),             # b_c per partition, per m2
    ("ba1",  1),             # [16, 1]
    ("ba24", 1),             # [128, 1]
    ("wseg4", 6),            # [k, b] w_seg per partition (x4 tiles)
    ("msel", 2),             # per-core batch select
    ("bmask", 12),           # [g, k] block mask
    ("bseg", 1),             # [12, 1] rows (g,k)
]
_WOFF = {}
_off = 0
for _n, _c in _WL:
    _WOFF[_n] = _off
    _off += _c
WCOLS = _off

TRACE = False
LAST_EXEC_NS = None
_CACHE = {}
# debug bisection: "stream" = matmul stream only (bd from bmask);
# "nocc" = head compute but no collectives (bd from bmask);
# "nowarm" = full minus warmup collective; "full" = everything
DEBUG = "full"


def _build_program():
    nc = bacc.Bacc("TRN2", target_bir_lowering=False, debug=False,
                   num_devices=N_CORES)

    pred_d = nc.dram_tensor("pred_s", [128, NG], bf16,
                            kind="ExternalInput").ap()
    xe_d = nc.dram_tensor("xe_par", [128, B * 1728], f32,
                          kind="ExternalInput").ap()
    wpack_d = nc.dram_tensor("wpack", [128, WCOLS], f32,
                             kind="ExternalInput").ap()
    out_d = nc.dram_tensor("out_s", [12, NG], f32,
                           kind="ExternalOutput").ap()

    with tile.TileContext(nc) as tc:
        with tc.tile_pool(name="small", bufs=1) as sp, \
             tc.tile_pool(name="pred", bufs=1) as pp, \
             tc.tile_pool(name="outp", bufs=2) as op, \
             tc.tile_pool(name="hps", bufs=3, space="PSUM") as hps, \
             tc.tile_pool(name="sps", bufs=4, space="PSUM") as sps, \
             tc.tile_pool(name="dram", bufs=1, space="DRAM") as dram:

            # packed weights + head x_e slab: first in the DMA queue
            wpack = sp.tile([128, WCOLS], f32)
            S = sp.tile([128, 16], f32)          # window sums (s, b)
            bd = sp.tile([128, 12], bf16)        # block-diagonal effw
            zs = sp.tile([128, 8], f32)          # z partials (m, b)
            xfs = sp.tile([128, 4], f32)         # xf partials (pc, b)
            p6T = sp.tile([128, 4 * 6], f32)     # (m, k, b)
            c6T = sp.tile([128, 2 * 6], f32)     # (m2, k, b)

            def W(name, p0=0, p1=128):
                c0 = _WOFF[name]
                c1 = c0 + dict(_WL)[name]
                return wpack[p0:p1, c0:c1]

            with tc.tile_pool(name="headbig", bufs=1) as hb, \
                 tc.tile_pool(name="scratch", bufs=2) as scp:

                xe = hb.tile([128, B * 1728], f32)
                nc.sync.dma_start(xe[:], xe_d[:])
                nc.sync.dma_start(wpack[:], wpack_d[:])

                # ---- collective-fabric warmup (no data deps) ----
                if DEBUG == "full":
                    warm_in = dram.tile([2, 2], f32)
                    warm_out = dram.tile([2, 2], f32)
                    nc.gpsimd.collective_compute(
                        "AllReduce", ALU.add,
                        replica_groups=[list(range(N_CORES))],
                        ins=[warm_in.opt()], outs=[warm_out.opt()])

                # ---- pred stream in 3 big chunks ----
                preds = pp.tile([128, NG], bf16)
                for ch in range(NCHUNK):
                    nc.sync.dma_start(preds[:, ch * CHW:(ch + 1) * CHW],
                                      pred_d[:, ch * CHW:(ch + 1) * CHW])

                if DEBUG == "stream":
                    nc.vector.tensor_copy(bd[:], W("bmask"))
                else:
                    _head(nc, tc, sp, hb, scp, hps, dram, W, xe, S, bd, zs,
                          xfs, p6T, c6T)

            # ---- main stream: 108 matmuls over resident pred ----
            bseg = W("bseg", 0, 12)[:, 0:1]
            for t in range(NITER):
                so = op.tile([12, COLS], f32, tag="so")
                for m in range(NMM):
                    c0 = t * COLS + m * 512
                    po = sps.tile([12, 512], f32, tag="po")
                    nc.tensor.matmul(po[:], bd[:], preds[:, c0:c0 + 512],
                                     start=True, stop=True)
                    if m % 2 == 0:
                        nc.scalar.activation(so[:, m * 512:(m + 1) * 512],
                                             po[:], AF.Identity, bias=bseg)
                    else:
                        nc.vector.tensor_scalar_add(
                            so[:, m * 512:(m + 1) * 512], po[:], bseg)
                nc.sync.dma_start(out_d[:, t * COLS:(t + 1) * COLS], so[:])

    nc.compile()
    return nc


def _head(nc, tc, sp, hb, scp, hps, dram, W, xe, S, bd, zs, xfs, p6T, c6T):
                # ---- GN stats (16 ch = 2 groups, core-local) ----
                # b=0 on scalar (ACT accum), b=1 on DVE: runs in parallel
                xe4 = xe[:].rearrange("p (b d w h) -> p b d w h",
                                      b=B, d=12, w=12, h=12)
                stat4 = sp.tile([128, 4], f32)  # cols: 2*b + (0=sum,1=sumsq)
                sl0 = xe[:, 0:1728]
                sl1 = xe[:, 1728:3456]
                sub = DEBUG.startswith("s:")
                want = DEBUG[2:] if sub else "abcd"
                if "a" in want:
                    st_sc = scp.tile([128, 1728], f32, tag="sc", bufs=2)
                    nc.scalar.activation(st_sc[:], sl0, AF.Copy,
                                         accum_out=stat4[:, 0:1])
                if "b" in want:
                    st2 = scp.tile([128, 1728], f32, tag="sc", bufs=2)
                    nc.scalar.activation(st2[:], sl0, AF.Square,
                                         accum_out=stat4[:, 1:2])
                if "c" in want:
                    nc.vector.tensor_reduce(stat4[:, 2:3], sl1,
                                            axis=AXL.X, op=ALU.add)
                if "d" in want:
                    st3 = scp.tile([128, 1728], f32, tag="sc", bufs=2)
                    nc.vector.tensor_mul(st3[:], sl1, sl1)
                    nc.vector.tensor_reduce(stat4[:, 3:4], st3[:],
                                            axis=AXL.X, op=ALU.add)

                if DEBUG == "stats" or sub:
                    nc.vector.tensor_copy(bd[:], W("bmask"))
                    return

                # group sums via mask matmul -> [2, 4]
                g4 = hps.tile([2, 4], f32, tag="hps")
                nc.tensor.matmul(g4[:], W("gstat"), stat4[:],
                                 start=True, stop=True)
                gsum = sp.tile([2, 4], f32)
                nc.vector.tensor_copy(gsum[:], g4[:])

                mr4 = sp.tile([2, 4], f32)  # cols: -mu0, -mu1, rs0, rs1
                nc.scalar.mul(mr4[:, 0:2], gsum[:, 0:4:2], -1.0 / NGRP_ELEMS)
                ex2 = sp.tile([2, 2], f32)
                nc.scalar.mul(ex2[:], gsum[:, 1:4:2], 1.0 / NGRP_ELEMS)
                musq = sp.tile([2, 2], f32)
                nc.vector.tensor_mul(musq[:], mr4[:, 0:2], mr4[:, 0:2])
                var = sp.tile([2, 2], f32)
                nc.vector.tensor_sub(var[:], ex2[:], musq[:])
                vare = sp.tile([2, 2], f32)
                nc.vector.tensor_scalar_add(vare[:], var[:], float(EPS))
                sd = sp.tile([2, 2], f32)
                nc.scalar.activation(sd[:], vare[:], AF.Sqrt)
                nc.vector.reciprocal(mr4[:, 2:4], sd[:])

                # expand groups -> partitions: chmr [128, 4]
                ch4 = hps.tile([128, 4], f32, tag="hps")
                nc.tensor.matmul(ch4[:], W("gexp", 0, 2), mr4[:],
                                 start=True, stop=True)
                chmr = sp.tile([128, 4], f32)
                nc.vector.tensor_copy(chmr[:], ch4[:])
                scale = sp.tile([128, 2], f32)
                nc.vector.tensor_scalar_mul(scale[:], chmr[:, 2:4],
                                            W("gnw")[:, 0:1])
                nmus = sp.tile([128, 2], f32)
                nc.vector.tensor_mul(nmus[:], chmr[:, 0:2], scale[:])
                bias = sp.tile([128, 2], f32)
                nc.vector.tensor_scalar_add(bias[:], nmus[:],
                                            W("gnw")[:, 1:2])

                if DEBUG == "mm":
                    nc.vector.tensor_copy(bd[:], W("bmask"))
                    return

                # ---- y = relu(scale*x + bias) (bf16) ----
                yt = hb.tile([128, B * 1728], bf16)
                for b in range(B):
                    nc.scalar.activation(yt[:, b * 1728:(b + 1) * 1728],
                                         xe[:, b * 1728:(b + 1) * 1728],
                                         AF.Relu, bias=bias[:, b:b + 1],
                                         scale=scale[:, b:b + 1])
                yt4 = yt[:].rearrange("p (b d w h) -> p b d w h",
                                      b=B, d=12, w=12, h=12)

                if DEBUG == "h1":
                    nc.vector.tensor_copy(bd[:], W("bmask"))
                    return

                # ---- window sums: 8 start-combos x 2 batches ----
                sidx = 0
                for sd_ in range(2):
                    for sw_ in range(2):
                        for sh_ in range(2):
                            for b in range(B):
                                col = 2 * sidx + b
                                win = yt4[:, b, sd_:sd_ + 11,
                                          sw_:sw_ + 11, sh_:sh_ + 11]
                                if col % 3 == 2:  # ~1/3 on scalar engine
                                    rl = scp.tile([128, 11 * 11 * 11], f32,
                                                  tag="rl", bufs=2)
                                    rl4 = rl[:].rearrange(
                                        "p (d w h) -> p d w h",
                                        d=11, w=11, h=11)
                                    nc.scalar.activation(
                                        rl4, win, AF.Copy,
                                        accum_out=S[:, col:col + 1])
                                else:
                                    nc.vector.tensor_reduce(
                                        S[:, col:col + 1], win,
                                        axis=AXL.XYZ, op=ALU.add)
                            sidx += 1

                if DEBUG == "h2":
                    nc.vector.tensor_copy(bd[:], W("bmask"))
                    return

                # ---- xf partials: [128, 4] cols (pc, b) ----
                w2 = W("w2dt").rearrange("p (s m c) -> p s m c",
                                         s=8, m=2, c=128)
                for m in range(2):
                    xfp = hps.tile([128, 2], f32, tag="hps")
                    for s in range(8):
                        nc.tensor.matmul(xfp[:], w2[:, s, m, :],
                                         S[:, 2 * s:2 * s + 2],
                                         start=(s == 0), stop=(s == 7))
                    nc.vector.tensor_copy(xfs[:, 2 * m:2 * m + 2], xfp[:])

                # ---- z partials: z[m] += WxT[pc,m] @ xfs[:, pc] ----
                wx = W("wx").rearrange("p (pc m c) -> p pc m c",
                                       pc=2, m=4, c=128)
                for m in range(4):
                    zp = hps.tile([128, 2], f32, tag="hps")
                    for pc in range(2):
                        nc.tensor.matmul(zp[:], wx[:, pc, m, :],
                                         xfs[:, 2 * pc:2 * pc + 2],
                                         start=(pc == 0), stop=(pc == 1))
                    nc.vector.tensor_copy(zs[:, 2 * m:2 * m + 2], zp[:])

                if DEBUG == "nocc":
                    nc.vector.tensor_copy(bd[:], W("bmask"))
                    return

                # ---- AllReduce z ----
                ar_in = dram.tile([128, 8], f32)
                ar_out = dram.tile([128, 8], f32)
                nc.gpsimd.dma_start(ar_in[:], zs[:])
                nc.gpsimd.collective_compute(
                    "AllReduce", ALU.add,
                    replica_groups=[list(range(N_CORES))],
                    ins=[ar_in.opt()], outs=[ar_out.opt()])
                zr = sp.tile([128, 8], f32)
                nc.gpsimd.dma_start(zr[:], ar_out[:])

                # ---- p = relu(z + bcf): [128, (m,k,b)] ----
                zr3 = zr[:].rearrange("p (m b) -> p m b", m=4, b=2)
                p63 = p6T[:].rearrange("p (m k b) -> p m k b", m=4, k=3, b=2)
                bcf3 = W("bcf").rearrange("p (m k b) -> p m k b",
                                          m=4, k=3, b=2)
                p6A = sp.tile([128, 4 * 6], f32)
                p6A3 = p6A[:].rearrange("p (m k b) -> p m k b", m=4, k=3, b=2)
                for k in range(K):
                    nc.vector.tensor_add(p6A3[:, :, k, :], zr3,
                                         bcf3[:, :, k, :])
                nc.scalar.activation(p6T[:], p6A[:], AF.Relu)

                # ---- c = Wc @ p + b_c: [128, (m2,k,b)] ----
                wc = W("wc").rearrange("p (pc m c) -> p pc m c",
                                       pc=4, m=2, c=128)
                p62 = p6T[:].rearrange("p (m kb) -> p m kb", m=4, kb=6)
                for m2 in range(2):
                    c1 = hps.tile([128, 6], f32, tag="hps")
                    for pc in range(4):
                        nc.tensor.matmul(c1[:], wc[:, pc, m2, :],
                                         p62[:, pc, :],
                                         start=(pc == 0), stop=(pc == 3))
                    nc.scalar.activation(c6T[:, 6 * m2:6 * m2 + 6], c1[:],
                                         AF.Identity,
                                         bias=W("bct")[:, m2:m2 + 1])

                # ---- h = relu(Wa1 @ c + b): [16, 6] ----
                wa1 = W("wa1").rearrange("p (pc c) -> p pc c", pc=2, c=16)
                h1 = hps.tile([16, 6], f32, tag="hps")
                for pc2 in range(2):
                    nc.tensor.matmul(h1[:], wa1[:, pc2, :],
                                     c6T[:, 6 * pc2:6 * pc2 + 6],
                                     start=(pc2 == 0), stop=(pc2 == 1))
                hT = sp.tile([16, 6], f32)
                nc.scalar.activation(hT[:], h1[:], AF.Relu,
                                     bias=W("ba1", 0, 16)[:, 0:1])

                # ---- gate = sigmoid(Wa2x4 @ h + b): [128, (k,b)] ----
                g1 = hps.tile([128, 6], f32, tag="hps")
                nc.tensor.matmul(g1[:], W("wa24", 0, 16), hT[:],
                                 start=True, stop=True)
                gT4 = sp.tile([128, 6], f32)
                nc.scalar.activation(gT4[:], g1[:], AF.Sigmoid,
                                     bias=W("ba24")[:, 0:1])

                # ---- effw + batch select + block-diagonal bd ----
                effT4 = sp.tile([128, 6], f32)
                nc.vector.tensor_mul(effT4[:], gT4[:], W("wseg4"))
                selL = sp.tile([128, 3], f32)
                nc.vector.tensor_scalar_mul(selL[:], effT4[:, 0:6:2],
                                            W("msel")[:, 0:1])
                selR = sp.tile([128, 3], f32)
                nc.vector.tensor_scalar_mul(selR[:], effT4[:, 1:6:2],
                                            W("msel")[:, 1:2])
                eff3 = sp.tile([128, 3], f32)
                nc.vector.tensor_add(eff3[:], selL[:], selR[:])
                for g in range(G):
                    nc.vector.tensor_mul(bd[:, 3 * g:3 * g + 3], eff3[:],
                                         W("bmask")[:, 3 * g:3 * g + 3])


def _prep_shared(inp):
    """Host-side weight packing shared by all cores (per-core pieces
    are filled in later)."""
    gap_b = np.asarray(inp["gap_b"], np.float32)
    w_cf = np.asarray(inp["w_cf"], np.float32)
    b_cf = np.asarray(inp["b_cf"], np.float32)
    w_c = np.asarray(inp["w_c"], np.float32)
    b_c = np.asarray(inp["b_c"], np.float32)
    w_a1 = np.asarray(inp["w_a1"], np.float32)
    b_a1 = np.asarray(inp["b_a1"], np.float32)
    w_a2 = np.asarray(inp["w_a2"], np.float32)
    b_a2 = np.asarray(inp["b_a2"], np.float32)
    emb = np.asarray(inp["emb"], np.float32)
    w_seg = np.asarray(inp["w_seg"], np.float32)
    b_seg = np.asarray(inp["b_seg"], np.float32)

    wp = np.zeros((128, WCOLS), np.float32)

    def put(name, arr, p0=0):
        c0 = _WOFF[name]
        arr = np.asarray(arr, np.float32)
        wp[p0:p0 + arr.shape[0], c0:c0 + arr.shape[1]] = arr

    # wx: [p, (pc, m, occ)] = WxT[pc*128+p, m*128+occ]
    wxT = w_cf[:, 0:256].T                        # [256, 512]
    put("wx", wxT.reshape(2, 128, 4, 128).transpose(1, 0, 2, 3)
        .reshape(128, -1))
    # wc: [p, (pc, m2, occ)] = WcT[pc*128+p, m2*128+occ]
    wcT = w_c.T                                   # [512, 256]
    put("wc", wcT.reshape(4, 128, 2, 128).transpose(1, 0, 2, 3)
        .reshape(128, -1))
    # wa1: [p, (pc2, o16)] = w_a1T[pc2*128+p, o]
    wa1T = w_a1.T                                 # [256, 16]
    put("wa1", wa1T.reshape(2, 128, 16).transpose(1, 0, 2).reshape(128, -1))
    # wa24: [16, 128] tiled gate head
    put("wa24", np.tile(w_a2.T, (1, 4)))          # [16, 128]
    # bcf: [p, (m, k, b)] = b_cf + We@emb[k] + Wx@gap_b  at oc=m*128+p
    bcfx = (b_cf[:, None] + w_cf[:, 256:512] @ emb.T
            + (wxT.T @ gap_b)[:, None])           # [512, 3]
    put("bcf", np.repeat(bcfx.reshape(4, 128, 3), 2, axis=2)
        .transpose(1, 0, 2).reshape(128, -1))
    p = np.arange(128)
    gstat = (p[:, None] // 64 == np.arange(2)[None, :]).astype(np.float32)
    put("gstat", gstat)
    put("gexp", gstat.T)                          # [2, 128]
    put("bct", np.repeat(b_c.reshape(2, 128).T, 1, axis=1))
    put("ba1", b_a1.reshape(16, 1))
    put("ba24", np.tile(b_a2, 4).reshape(128, 1))
    # wseg4: [p, (k, b)] = w_seg[k, p%32]
    put("wseg4", np.repeat(np.tile(w_seg.T, (4, 1)), 2, axis=1))
    bmask = (p[:, None] // 32 == np.arange(4)[None, :]).astype(np.float32)
    put("bmask", np.repeat(bmask, 3, axis=1))
    put("bseg", np.tile(b_seg, 4).reshape(12, 1))
    return wp


def kernel(**inputs):
    global LAST_EXEC_NS
    x_e = np.asarray(inputs["x_e"], np.float32)
    pred = np.asarray(inputs["pred"], np.float32)
    gap_w = np.asarray(inputs["gap_w"], np.float32)
    gn_g = np.asarray(inputs["gn_g"], np.float32)
    gn_b = np.asarray(inputs["gn_b"], np.float32)

    wp_shared = _prep_shared(inputs)
    w2 = gap_w.reshape(256, 128, 3, 3, 3)
    pred_bf = pred.astype(bfloat16)

    # parity-packed x_e: [B, 128ch, (pd,dl), (pw,wl), (ph,hl)]
    xe_p = x_e.reshape(B, 128, 12, 2, 12, 2, 12, 2)

    in_maps = []
    for r in range(N_CORES):
        b, dq = divmod(r, 4)
        ch = slice(16 * r, 16 * r + 16)

        ps = pred_bf[b, :, dq * 24:(dq + 1) * 24]       # [32, 24, 96, 96]
        ps = ps.reshape(32, G, NG).transpose(1, 0, 2)
        m = {"pred_s": np.ascontiguousarray(ps.reshape(128, NG)),
             "wpack": wp_shared.copy()}

        # xe_par: partitions (c:16, pd, pw, ph), cols (b, dl, wl, hl)
        sl = xe_p[:, ch]                                # [B,16,12,2,12,2,12,2]
        m["xe_par"] = np.ascontiguousarray(
            sl.transpose(1, 3, 5, 7, 0, 2, 4, 6).reshape(128, -1))

        # w2dt: [p=(c,par), (s, m, occ)] = W2[m*128+occ, ch(c), o]/1331
        # where o = (2*sd+pd, 2*sw+pw, 2*sh+ph) if valid else 0
        w2c = w2[:, ch]                                 # [256, 16, 3,3,3]
        w2dt = np.zeros((16, 2, 2, 2, 8, 2, 128), np.float32)
        for sdi in range(2):
            for swi in range(2):
                for shi in range(2):
                    s = 4 * sdi + 2 * swi + shi
                    for pd in range(2):
                        kd = 2 * sdi + pd
                        if kd > 2:
                            continue
                        for pw in range(2):
                            kw = 2 * swi + pw
                            if kw > 2:
                                continue
                            for ph in range(2):
                                kh = 2 * shi + ph
                                if kh > 2:
                                    continue
                                par = 4 * pd + 2 * pw + ph
                                blk = w2c[:, :, kd, kw, kh].T / 1331.0
                                w2dt[:, pd, pw, ph, s] = \
                                    blk.reshape(16, 2, 128)
        c0 = _WOFF["w2dt"]
        m["wpack"][:, c0:c0 + 2048] = w2dt.reshape(128, -1)

        gnwc = np.stack([np.repeat(gn_g[ch], 8), np.repeat(gn_b[ch], 8)],
                        axis=1)
        c0 = _WOFF["gnw"]
        m["wpack"][:, c0:c0 + 2] = gnwc

        msel = np.zeros((128, 2), np.float32)
        msel[:, b] = 1.0
        c0 = _WOFF["msel"]
        m["wpack"][:, c0:c0 + 2] = msel
        in_maps.append(m)

    if DEBUG not in _CACHE:
        _CACHE[DEBUG] = _build_program()
    nc = _CACHE[DEBUG]

    res = run_bass_kernel_spmd(nc, in_maps, list(range(N_CORES)),
                               trace=TRACE)
    LAST_EXEC_NS = res.exec_time_ns

    out = np.empty((B, K, 96, 96, 96), np.float32)
    for r in range(N_CORES):
        b, dq = divmod(r, 4)
        o = res.results[r]["out_s"]                      # [12, NG]
        o = o.reshape(G, K, NG).transpose(1, 0, 2).reshape(K, NPOS)
        out[b, :, dq * 24:(dq + 1) * 24] = o.reshape(K, 24, 96, 96)
    return out
